# revision 10
# baseline (speedup 1.0000x reference)
"""Trainium2 Bass kernel for nn_Attention_Pooling_GNN (segment_reduce).

Strategy (data-parallel by graph, 8 cores):
  - Each core owns 4 of the 32 batch graphs. `batch` is sorted, so each
    graph's nodes are a contiguous row range of x. The host packs the 4
    graphs into 4 fixed-size row slots of NG rows each (zero rows pad the
    tail of a slot; zero rows add nothing to any segment), so the slot ->
    graph mapping is a compile-time constant shared by all 8 cores (SPMD).
  - On device, the segment sum is a one-hot matmul on the tensor engine:
    for every 128-node chunk, onehot[p, t] = (patch[p] == t) over only the
    36 spatial cells (the graph is known from the slot), built 8 chunks at
    a time by one vector-engine is_equal, then
    psum[C, 36*slot : 36*slot+36] += x_chunk.T @ onehot.
  - The tiny attention head (W_emb/W_att/W_fc + softmax over 36 cells) runs
    per-core on its own 4 graphs; the host concatenates the per-core [4, 2]
    logits into the final [32, 2].
"""

import ml_dtypes
import numpy as np

from concourse import bacc, mybir
from concourse.tile import TileContext
from concourse.bass_utils import run_bass_kernel_spmd

# Problem constants (must match reference.setup_inputs()).
N = 200000
C = 128
H = 128
OUT = 2
B = 32
GRID = 1216
PATCH = 202
NP_ = 6             # patches per axis
NCELLS = NP_ * NP_  # 36
M = 8               # cores
BL = B // M         # graphs per core = 4
S = NCELLS * BL     # 144 segments per core
P = 128

SLAB = 16           # chunks per x DMA (1 MiB)
BUILD_K = 8         # chunks whose one-hots are built by one vector op

F32 = mybir.dt.float32
BF16 = mybir.dt.bfloat16

# x representation: "f32" exact (slow fp32 matmuls), "bf16" (cast during
# DMA, ~8e-3 rel err), or "hilo" (host splits x into bf16 hi+lo halves --
# same HBM bytes as fp32, ~1e-5 rel err, bf16-speed matmuls)
X_MODE = "hilo"

_cache: dict = {}


def _build(ng: int, repeat: int = 1):
    """Build the per-core program. `repeat` > 1 wraps the whole body in an
    on-device loop - used only for timing (amortizes launch overhead)."""
    ngc = ng // P            # chunks per graph slot
    nchunks = BL * ngc       # chunks per graph input
    nrows = BL * ng
    nc = bacc.Bacc("TRN2", target_bir_lowering=False, debug=False, num_devices=M)

    xw = 2 * C if X_MODE == "hilo" else C
    xdrdt = BF16 if X_MODE == "hilo" else F32
    x1 = nc.declare_dram_parameter("x1", [nrows, xw], xdrdt, isOutput=False)
    x2 = nc.declare_dram_parameter("x2", [nrows, xw], xdrdt, isOutput=False)
    seg1 = nc.declare_dram_parameter("seg1", [P, nchunks], F32, isOutput=False)
    seg2 = nc.declare_dram_parameter("seg2", [P, nchunks], F32, isOutput=False)
    iota_in = nc.declare_dram_parameter("iota", [P, BUILD_K * NCELLS], F32,
                                        isOutput=False)
    wemb = nc.declare_dram_parameter("wemb", [2 * C, H], F32, isOutput=False)
    bemb = nc.declare_dram_parameter("bemb", [H, 1], F32, isOutput=False)
    watt = nc.declare_dram_parameter("watt", [H, 1], F32, isOutput=False)
    batt = nc.declare_dram_parameter("batt", [1, 1], F32, isOutput=False)
    wfc = nc.declare_dram_parameter("wfc", [H, OUT], F32, isOutput=False)
    bfc = nc.declare_dram_parameter("bfc", [1, OUT], F32, isOutput=False)
    ident = nc.declare_dram_parameter("ident", [P, P], F32, isOutput=False)
    out = nc.declare_dram_parameter("out", [BL, OUT], F32, isOutput=True)

    with TileContext(nc) as tc:
        with (
            tc.tile_pool(name="const", bufs=1) as cpool,
            tc.tile_pool(name="xs", bufs=3) as xpool,
            tc.tile_pool(name="oh", bufs=4) as ohpool,
            tc.tile_pool(name="head", bufs=1) as hpool,
            tc.tile_pool(name="psum", bufs=1, space="PSUM") as ppool,
        ):
            iota_sb = cpool.tile([P, BUILD_K * NCELLS], F32, tag="iota")
            nc.sync.dma_start(out=iota_sb[:], in_=iota_in[:])
            seg_sb = [cpool.tile([P, nchunks], F32, tag=f"seg{g}",
                                 name=f"seg{g}_sb") for g in (1, 2)]
            nc.sync.dma_start(out=seg_sb[0][:], in_=seg1[:])
            nc.sync.dma_start(out=seg_sb[1][:], in_=seg2[:])

            wemb_sb = cpool.tile([P, 2 * C], F32, tag="wemb")
            nc.sync.dma_start(out=wemb_sb[:, 0:C], in_=wemb[0:C, :])
            nc.sync.dma_start(out=wemb_sb[:, C : 2 * C], in_=wemb[C : 2 * C, :])
            bemb_sb = cpool.tile([P, 1], F32, tag="bemb")
            nc.sync.dma_start(out=bemb_sb[:], in_=bemb[:])
            watt_sb = cpool.tile([P, 1], F32, tag="watt")
            nc.sync.dma_start(out=watt_sb[:], in_=watt[:])
            batt_sb = cpool.tile([1, 1], F32, tag="batt")
            nc.sync.dma_start(out=batt_sb[:], in_=batt[:])
            wfc_sb = cpool.tile([P, OUT], F32, tag="wfc")
            nc.sync.dma_start(out=wfc_sb[:], in_=wfc[:])
            bfc_sb = cpool.tile([1, OUT], F32, tag="bfc")
            nc.sync.dma_start(out=bfc_sb[:], in_=bfc[:])
            ones_sb = cpool.tile([1, P], F32, tag="ones")
            nc.vector.memset(ones_sb[:], 1.0)
            ident_sb = cpool.tile([P, P], F32, tag="ident")
            nc.sync.dma_start(out=ident_sb[:], in_=ident[:])

            def body():
                if X_MODE == "hilo":
                    # [cell, slot*(hi|lo)*C] partial sums; combined+transposed
                    # into [C, S] after the loop
                    pool_ps = [
                        ppool.tile([NCELLS, BL * 2 * C], F32, tag=t,
                                   space="PSUM", name=f"pool_ps{t}")
                        for t in ("psA", "psB")
                    ]
                else:
                    pool_ps = [
                        ppool.tile([P, S], F32, tag=t, space="PSUM",
                                   name=f"pool_ps{t}")
                        for t in ("psA", "psB")
                    ]

                # ---- segment-sum ----
                for gi, xin in enumerate((x1, x2)):
                    xv = xin[:].rearrange("(n p) c -> p n c", p=P)
                    ps = pool_ps[gi]
                    oh = None
                    for s0 in range(0, nchunks, SLAB):
                        cnt = min(SLAB, nchunks - s0)
                        xdt = F32 if X_MODE == "f32" else BF16
                        xt = xpool.tile([P, SLAB * xw], xdt, tag="xslab")
                        dma_eng = nc.gpsimd if X_MODE == "bf16" else nc.sync
                        dma_eng.dma_start(
                            out=xt[:, : cnt * xw].rearrange("p (n c) -> p n c", c=xw),
                            in_=xv[:, s0 : s0 + cnt, :],
                        )
                        for j in range(cnt):
                            ci = s0 + j
                            bj = ci % BUILD_K
                            if bj == 0:
                                # one is_equal builds one-hots for BUILD_K
                                # chunks: seg col broadcast over the 36 cells
                                bk = min(BUILD_K, nchunks - ci)
                                oh = ohpool.tile(
                                    [P, BUILD_K * NCELLS],
                                    F32 if X_MODE == "f32" else BF16,
                                    tag="oh", name="oh")
                                nc.vector.tensor_tensor(
                                    out=oh[:, : bk * NCELLS].rearrange(
                                        "p (k t) -> p k t", t=NCELLS),
                                    in0=iota_sb[:, : bk * NCELLS].rearrange(
                                        "p (k t) -> p k t", t=NCELLS),
                                    in1=seg_sb[gi][:, ci : ci + bk, None]
                                        .to_broadcast([P, bk, NCELLS]),
                                    op=mybir.AluOpType.is_equal,
                                )
                            slot = ci // ngc
                            loc = ci % ngc
                            if X_MODE == "hilo":
                                # out[cell, (hi|lo)*C] += onehot.T @ [xhi|xlo]
                                nc.tensor.matmul(
                                    out=ps[0:NCELLS,
                                           slot * 2 * C : (slot + 1) * 2 * C],
                                    lhsT=oh[:, bj * NCELLS : (bj + 1) * NCELLS],
                                    rhs=xt[:, j * xw : (j + 1) * xw],
                                    start=(loc == 0),
                                    stop=(loc == ngc - 1),
                                )
                            else:
                                nc.tensor.matmul(
                                    out=ps[:, slot * NCELLS : (slot + 1) * NCELLS],
                                    lhsT=xt[:, j * C : (j + 1) * C],
                                    rhs=oh[:, bj * NCELLS : (bj + 1) * NCELLS],
                                    start=(loc == 0),
                                    stop=(loc == ngc - 1),
                                )

                # ---- head: embT[H, S] = W_emb.T @ pooled (+ b_emb) ----
                pooled_sb = [hpool.tile([P, S], F32, tag=f"pooled{g}",
                                        name=f"pooled{g}_sb") for g in (1, 2)]
                if X_MODE == "hilo":
                    # hi+lo recombine, then PE-transpose [cell, C] -> [C, cell]
                    pooledT_ps = [
                        ppool.tile([P, S], F32, tag=t, space="PSUM",
                                   name=f"pooledT_{t}")
                        for t in ("psC", "psD")
                    ]
                    for gi in range(2):
                        ps36_sb = hpool.tile([NCELLS, BL * 2 * C], F32,
                                             tag="ps36_sb", name="ps36_sb")
                        nc.vector.tensor_copy(out=ps36_sb[:],
                                              in_=pool_ps[gi][:])
                        comb = hpool.tile([NCELLS, BL * C], F32, tag="comb",
                                          name="comb")
                        v = ps36_sb.rearrange("t (k h c) -> t k h c", h=2, c=C)
                        nc.vector.tensor_tensor(
                            out=comb[:].rearrange("t (k c) -> t k c", c=C),
                            in0=v[:, :, 0, :], in1=v[:, :, 1, :],
                            op=mybir.AluOpType.add,
                        )
                        for sl in range(BL):
                            nc.tensor.transpose(
                                out=pooledT_ps[gi][:, sl * NCELLS
                                                   : (sl + 1) * NCELLS],
                                in_=comb[0:NCELLS, sl * C : (sl + 1) * C],
                                identity=ident_sb[0:NCELLS, 0:NCELLS],
                            )
                        nc.vector.tensor_copy(out=pooled_sb[gi][:],
                                              in_=pooledT_ps[gi][:])
                else:
                    nc.vector.tensor_copy(out=pooled_sb[0][:], in_=pool_ps[0][:])
                    nc.vector.tensor_copy(out=pooled_sb[1][:], in_=pool_ps[1][:])

                emb_ps = ppool.tile([P, S], F32, tag="psA", space="PSUM", name="emb_ps")
                nc.tensor.matmul(out=emb_ps[:], lhsT=wemb_sb[:, 0:C],
                                 rhs=pooled_sb[0][:], start=True, stop=False)
                nc.tensor.matmul(out=emb_ps[:], lhsT=wemb_sb[:, C : 2 * C],
                                 rhs=pooled_sb[1][:], start=False, stop=True)
                embT_sb = hpool.tile([P, S], F32, tag="embT")
                nc.vector.tensor_scalar(
                    out=embT_sb[:], in0=emb_ps[:], scalar1=bemb_sb[:, 0:1],
                    scalar2=None, op0=mybir.AluOpType.add,
                )

                # ---- attention logits z = embT.T @ W_att + b_att -> [1, S] ----
                z_ps = ppool.tile([1, S], F32, tag="psB", space="PSUM", name="z_ps")
                nc.tensor.matmul(out=z_ps[:], lhsT=watt_sb[:], rhs=embT_sb[:],
                                 start=True, stop=True)
                z_sb = hpool.tile([1, S], F32, tag="z")
                nc.vector.tensor_scalar(
                    out=z_sb[:], in0=z_ps[:], scalar1=batt_sb[0:1, 0:1],
                    scalar2=None, op0=mybir.AluOpType.add,
                )

                # ---- per-graph softmax over the 36 cells ----
                att_sb = hpool.tile([1, S], F32, tag="att")
                mx_sb = hpool.tile([1, BL], F32, tag="mx")
                nmx_sb = hpool.tile([1, BL], F32, tag="nmx")
                ssum_sb = hpool.tile([1, BL], F32, tag="ssum")
                rcp_sb = hpool.tile([1, BL], F32, tag="rcp")
                for g in range(BL):
                    sl = slice(g * NCELLS, (g + 1) * NCELLS)
                    nc.vector.tensor_reduce(
                        out=mx_sb[0:1, g : g + 1], in_=z_sb[0:1, sl],
                        axis=mybir.AxisListType.X, op=mybir.AluOpType.max,
                    )
                    nc.vector.tensor_scalar(
                        out=nmx_sb[0:1, g : g + 1], in0=mx_sb[0:1, g : g + 1],
                        scalar1=-1.0, scalar2=None, op0=mybir.AluOpType.mult,
                    )
                    nc.scalar.activation(
                        out=att_sb[0:1, sl], in_=z_sb[0:1, sl],
                        func=mybir.ActivationFunctionType.Exp,
                        bias=nmx_sb[0:1, g : g + 1], scale=1.0,
                        accum_out=ssum_sb[0:1, g : g + 1],
                    )
                    nc.vector.reciprocal(rcp_sb[0:1, g : g + 1],
                                         ssum_sb[0:1, g : g + 1])
                    nc.vector.tensor_scalar(
                        out=att_sb[0:1, sl], in0=att_sb[0:1, sl],
                        scalar1=rcp_sb[0:1, g : g + 1], scalar2=None,
                        op0=mybir.AluOpType.mult,
                    )

                # ---- attended[H, G] = sum_cells att * embT ----
                attb_ps = ppool.tile([P, S], F32, tag="psC", space="PSUM", name="attb_ps")
                nc.tensor.matmul(out=attb_ps[:], lhsT=ones_sb[0:1, 0:P],
                                 rhs=att_sb[0:1, :], start=True, stop=True)
                wsum_sb = hpool.tile([P, S], F32, tag="wsum")
                nc.vector.tensor_tensor(
                    out=wsum_sb[:], in0=embT_sb[:], in1=attb_ps[:],
                    op=mybir.AluOpType.mult,
                )
                attd_sb = hpool.tile([P, BL], F32, tag="attd")
                for g in range(BL):
                    nc.vector.tensor_reduce(
                        out=attd_sb[:, g : g + 1],
                        in_=wsum_sb[:, g * NCELLS : (g + 1) * NCELLS],
                        axis=mybir.AxisListType.X, op=mybir.AluOpType.add,
                    )

                # ---- logits = attended.T @ W_fc + b_fc -> [G, OUT] ----
                log_ps = ppool.tile([BL, OUT], F32, tag="psD", space="PSUM", name="log_ps")
                nc.tensor.matmul(out=log_ps[:], lhsT=attd_sb[:, 0:BL],
                                 rhs=wfc_sb[:], start=True, stop=False)
                nc.tensor.matmul(out=log_ps[:], lhsT=ones_sb[0:1, 0:BL],
                                 rhs=bfc_sb[0:1, :], start=False, stop=True,
                                 skip_group_check=True)
                out_sb = hpool.tile([BL, OUT], F32, tag="out")
                nc.vector.tensor_copy(out=out_sb[:], in_=log_ps[:])
                nc.sync.dma_start(out=out[:], in_=out_sb[:])

            if repeat == 1:
                body()
            else:
                with tc.For_i(0, repeat, 1):
                    body()

    nc.compile()
    return nc


def _get(ng: int):
    if ng not in _cache:
        _cache[ng] = _build(ng)
    return _cache[ng]


def _patch_ids(pos: np.ndarray) -> np.ndarray:
    px = np.clip(np.floor(pos[:, 0] / np.float32(PATCH)), 0, NP_ - 1).astype(np.int32)
    py = np.clip(np.floor(pos[:, 1] / np.float32(PATCH)), 0, NP_ - 1).astype(np.int32)
    return px * NP_ + py


def prepare(x_graph_1, x_graph_2, pos_graph_1, pos_graph_2,
            batch_graph_1, batch_graph_2,
            W_emb, b_emb, W_att, b_att, W_fc, b_fc):
    """Compute per-core input maps + the slot size NG. Host-side only."""
    graphs = []
    for x, pos, batch in ((x_graph_1, pos_graph_1, batch_graph_1),
                          (x_graph_2, pos_graph_2, batch_graph_2)):
        x = np.asarray(x, dtype=np.float32)
        pos = np.asarray(pos, dtype=np.float32)
        batch = np.asarray(batch, dtype=np.int32)
        patch = _patch_ids(pos).astype(np.float32)
        bounds = np.searchsorted(batch, np.arange(B + 1)).astype(np.int64)
        if X_MODE == "hilo":
            xhi = x.astype(ml_dtypes.bfloat16)
            xlo = (x - xhi.astype(np.float32)).astype(ml_dtypes.bfloat16)
            x = (xhi, xlo)
        graphs.append((x, patch, bounds))

    span = max(int(np.diff(g[2]).max()) for g in graphs)
    ng = max(-(-span // P) * P, 2 * P)
    ngc = ng // P
    nchunks = BL * ngc
    nrows = BL * ng

    iota_tile = np.tile(np.arange(NCELLS, dtype=np.float32), (P, BUILD_K))
    ident_tile = np.eye(P, dtype=np.float32)
    w_emb = np.asarray(W_emb, dtype=np.float32)
    b_emb2 = np.asarray(b_emb, dtype=np.float32).reshape(H, 1)
    w_att = np.asarray(W_att, dtype=np.float32).reshape(H, 1)
    b_att2 = np.asarray(b_att, dtype=np.float32).reshape(1, 1)
    w_fc = np.asarray(W_fc, dtype=np.float32).reshape(H, OUT)
    b_fc2 = np.asarray(b_fc, dtype=np.float32).reshape(1, OUT)

    in_maps = []
    for k in range(M):
        im = {"iota": iota_tile, "ident": ident_tile, "wemb": w_emb,
              "bemb": b_emb2, "watt": w_att, "batt": b_att2, "wfc": w_fc,
              "bfc": b_fc2}
        for gi, (x, patch, bounds) in enumerate(graphs):
            if X_MODE == "hilo":
                xhi, xlo = x
                xbuf = np.zeros((nrows, 2 * C), dtype=ml_dtypes.bfloat16)
            else:
                xbuf = np.zeros((nrows, C), dtype=np.float32)
            segbuf = np.zeros(nrows, dtype=np.float32)
            for g in range(BL):
                lo = int(bounds[k * BL + g])
                hi = int(bounds[k * BL + g + 1])
                if X_MODE == "hilo":
                    xbuf[g * ng : g * ng + (hi - lo), 0:C] = xhi[lo:hi]
                    xbuf[g * ng : g * ng + (hi - lo), C : 2 * C] = xlo[lo:hi]
                else:
                    xbuf[g * ng : g * ng + (hi - lo)] = x[lo:hi]
                segbuf[g * ng : g * ng + (hi - lo)] = patch[lo:hi]
            im[f"x{gi + 1}"] = xbuf
            im[f"seg{gi + 1}"] = np.ascontiguousarray(
                segbuf.reshape(nchunks, P).T)
        in_maps.append(im)
    return ng, in_maps


def kernel(**inputs) -> np.ndarray:
    ng, in_maps = prepare(**inputs)
    nc = _get(ng)
    res = run_bass_kernel_spmd(nc, in_maps, list(range(M)))
    return np.concatenate([res.results[k]["out"] for k in range(M)], axis=0)


# revision 11
# speedup vs baseline: 1.0416x; 1.0416x over previous
"""Trainium2 Bass kernel for nn_Attention_Pooling_GNN (segment_reduce).

Strategy (data-parallel by graph, 8 cores):
  - Each core owns 4 of the 32 batch graphs. `batch` is sorted, so each
    graph's nodes are a contiguous row range of x. The host packs the 4
    graphs into 4 fixed-size row slots of NG rows each (zero rows pad the
    tail of a slot; zero rows add nothing to any segment), so the slot ->
    graph mapping is a compile-time constant shared by all 8 cores (SPMD).
  - On device, the segment sum is a one-hot matmul on the tensor engine:
    for every 128-node chunk, onehot[p, t] = (patch[p] == t) over only the
    36 spatial cells (the graph is known from the slot), built 8 chunks at
    a time by one vector-engine is_equal, then
    psum[C, 36*slot : 36*slot+36] += x_chunk.T @ onehot.
  - The tiny attention head (W_emb/W_att/W_fc + softmax over 36 cells) runs
    per-core on its own 4 graphs; the host concatenates the per-core [4, 2]
    logits into the final [32, 2].
"""

import ml_dtypes
import numpy as np

from concourse import bacc, mybir
from concourse.tile import TileContext
from concourse.bass_utils import run_bass_kernel_spmd

# Problem constants (must match reference.setup_inputs()).
N = 200000
C = 128
H = 128
OUT = 2
B = 32
GRID = 1216
PATCH = 202
NP_ = 6             # patches per axis
NCELLS = NP_ * NP_  # 36
M = 8               # cores
BL = B // M         # graphs per core = 4
S = NCELLS * BL     # 144 segments per core
P = 128

SLAB = 16           # chunks per x DMA (1 MiB)
BUILD_K = 8         # chunks whose one-hots are built by one vector op

F32 = mybir.dt.float32
BF16 = mybir.dt.bfloat16

# x representation: "f32" exact (slow fp32 matmuls), "bf16" (cast during
# DMA, ~8e-3 rel err), or "hilo" (host splits x into bf16 hi+lo halves --
# same HBM bytes as fp32, ~1e-5 rel err, bf16-speed matmuls)
X_MODE = "hilo"

_cache: dict = {}


def _build(ng: int, repeat: int = 1):
    """Build the per-core program. `repeat` > 1 wraps the whole body in an
    on-device loop - used only for timing (amortizes launch overhead)."""
    ngc = ng // P            # chunks per graph slot
    nchunks = BL * ngc       # chunks per graph input
    nrows = BL * ng
    nc = bacc.Bacc("TRN2", target_bir_lowering=False, debug=False, num_devices=M)

    xw = 2 * C if X_MODE == "hilo" else C
    xdrdt = BF16 if X_MODE == "hilo" else F32
    x1 = nc.declare_dram_parameter("x1", [nrows, xw], xdrdt, isOutput=False)
    x2 = nc.declare_dram_parameter("x2", [nrows, xw], xdrdt, isOutput=False)
    seg1 = nc.declare_dram_parameter("seg1", [P, nchunks], F32, isOutput=False)
    seg2 = nc.declare_dram_parameter("seg2", [P, nchunks], F32, isOutput=False)
    iota_in = nc.declare_dram_parameter("iota", [P, BUILD_K * NCELLS], F32,
                                        isOutput=False)
    wemb = nc.declare_dram_parameter("wemb", [2 * C, H], F32, isOutput=False)
    bemb = nc.declare_dram_parameter("bemb", [H, 1], F32, isOutput=False)
    watt = nc.declare_dram_parameter("watt", [H, 1], F32, isOutput=False)
    batt = nc.declare_dram_parameter("batt", [1, 1], F32, isOutput=False)
    wfc = nc.declare_dram_parameter("wfc", [H, OUT], F32, isOutput=False)
    bfc = nc.declare_dram_parameter("bfc", [1, OUT], F32, isOutput=False)
    ident = nc.declare_dram_parameter("ident", [P, P], F32, isOutput=False)
    out = nc.declare_dram_parameter("out", [BL, OUT], F32, isOutput=True)

    with TileContext(nc) as tc:
        with (
            tc.tile_pool(name="const", bufs=1) as cpool,
            tc.tile_pool(name="xs", bufs=3) as xpool,
            tc.tile_pool(name="oh", bufs=4) as ohpool,
            tc.tile_pool(name="head", bufs=1) as hpool,
            tc.tile_pool(name="psum", bufs=1, space="PSUM") as ppool,
        ):
            iota_sb = cpool.tile([P, BUILD_K * NCELLS], F32, tag="iota")
            nc.sync.dma_start(out=iota_sb[:], in_=iota_in[:])
            seg_sb = [cpool.tile([P, nchunks], F32, tag=f"seg{g}",
                                 name=f"seg{g}_sb") for g in (1, 2)]
            nc.sync.dma_start(out=seg_sb[0][:], in_=seg1[:])
            nc.sync.dma_start(out=seg_sb[1][:], in_=seg2[:])

            wemb_sb = cpool.tile([P, 2 * C], F32, tag="wemb")
            nc.sync.dma_start(out=wemb_sb[:, 0:C], in_=wemb[0:C, :])
            nc.sync.dma_start(out=wemb_sb[:, C : 2 * C], in_=wemb[C : 2 * C, :])
            bemb_sb = cpool.tile([P, 1], F32, tag="bemb")
            nc.sync.dma_start(out=bemb_sb[:], in_=bemb[:])
            watt_sb = cpool.tile([P, 1], F32, tag="watt")
            nc.sync.dma_start(out=watt_sb[:], in_=watt[:])
            batt_sb = cpool.tile([1, 1], F32, tag="batt")
            nc.sync.dma_start(out=batt_sb[:], in_=batt[:])
            wfc_sb = cpool.tile([P, OUT], F32, tag="wfc")
            nc.sync.dma_start(out=wfc_sb[:], in_=wfc[:])
            bfc_sb = cpool.tile([1, OUT], F32, tag="bfc")
            nc.sync.dma_start(out=bfc_sb[:], in_=bfc[:])
            ones_sb = cpool.tile([1, P], F32, tag="ones")
            nc.vector.memset(ones_sb[:], 1.0)
            ident_sb = cpool.tile([P, P], F32, tag="ident")
            nc.sync.dma_start(out=ident_sb[:], in_=ident[:])

            def body():
                if X_MODE == "hilo":
                    # [cell-band, slot*(hi|lo)*C] partial sums: even chunks
                    # accumulate into partitions 0:36 (PE col-group 0), odd
                    # chunks into partitions 64:100 (col-group 64) so pairs of
                    # matmuls run concurrently; combined + transposed into
                    # [C, S] after the loop
                    pool_ps = [
                        ppool.tile([P, BL * 2 * C], F32, tag=t,
                                   space="PSUM", name=f"pool_ps{t}")
                        for t in ("psA", "psB")
                    ]
                else:
                    pool_ps = [
                        ppool.tile([P, S], F32, tag=t, space="PSUM",
                                   name=f"pool_ps{t}")
                        for t in ("psA", "psB")
                    ]

                # ---- segment-sum ----
                for gi, xin in enumerate((x1, x2)):
                    xv = xin[:].rearrange("(n p) c -> p n c", p=P)
                    ps = pool_ps[gi]
                    oh = None
                    for s0 in range(0, nchunks, SLAB):
                        cnt = min(SLAB, nchunks - s0)
                        xdt = F32 if X_MODE == "f32" else BF16
                        xt = xpool.tile([P, SLAB * xw], xdt, tag="xslab")
                        dma_eng = nc.gpsimd if X_MODE == "bf16" else nc.sync
                        dma_eng.dma_start(
                            out=xt[:, : cnt * xw].rearrange("p (n c) -> p n c", c=xw),
                            in_=xv[:, s0 : s0 + cnt, :],
                        )
                        for j in range(cnt):
                            ci = s0 + j
                            bj = ci % BUILD_K
                            if bj == 0:
                                # one is_equal builds one-hots for BUILD_K
                                # chunks: seg col broadcast over the 36 cells
                                bk = min(BUILD_K, nchunks - ci)
                                oh = ohpool.tile(
                                    [P, BUILD_K * NCELLS],
                                    F32 if X_MODE == "f32" else BF16,
                                    tag="oh", name="oh")
                                nc.vector.tensor_tensor(
                                    out=oh[:, : bk * NCELLS].rearrange(
                                        "p (k t) -> p k t", t=NCELLS),
                                    in0=iota_sb[:, : bk * NCELLS].rearrange(
                                        "p (k t) -> p k t", t=NCELLS),
                                    in1=seg_sb[gi][:, ci : ci + bk, None]
                                        .to_broadcast([P, bk, NCELLS]),
                                    op=mybir.AluOpType.is_equal,
                                )
                            slot = ci // ngc
                            loc = ci % ngc
                            if X_MODE == "hilo":
                                # out[cell, (hi|lo)*C] += onehot.T @ [xhi|xlo]
                                band = 64 * (loc % 2)
                                last = ngc - 1 - ((ngc - 1 - loc % 2) % 2)
                                nc.tensor.matmul(
                                    out=ps[band : band + NCELLS,
                                           slot * 2 * C : (slot + 1) * 2 * C],
                                    lhsT=oh[:, bj * NCELLS : (bj + 1) * NCELLS],
                                    rhs=xt[:, j * xw : (j + 1) * xw],
                                    start=(loc == loc % 2),
                                    stop=(loc == last),
                                    tile_position=(0, band),
                                )
                            else:
                                nc.tensor.matmul(
                                    out=ps[:, slot * NCELLS : (slot + 1) * NCELLS],
                                    lhsT=xt[:, j * C : (j + 1) * C],
                                    rhs=oh[:, bj * NCELLS : (bj + 1) * NCELLS],
                                    start=(loc == 0),
                                    stop=(loc == ngc - 1),
                                )

                # ---- head: embT[H, S] = W_emb.T @ pooled (+ b_emb) ----
                pooled_sb = [hpool.tile([P, S], F32, tag=f"pooled{g}",
                                        name=f"pooled{g}_sb") for g in (1, 2)]
                if X_MODE == "hilo":
                    # hi+lo recombine, then PE-transpose [cell, C] -> [C, cell]
                    pooledT_ps = [
                        ppool.tile([P, S], F32, tag=t, space="PSUM",
                                   name=f"pooledT_{t}")
                        for t in ("psC", "psD")
                    ]
                    for gi in range(2):
                        ps36_sb = hpool.tile([P, BL * 2 * C], F32,
                                             tag="ps36_sb", name="ps36_sb")
                        nc.vector.tensor_copy(out=ps36_sb[:],
                                              in_=pool_ps[gi][:])
                        comb = hpool.tile([NCELLS, BL * C], F32, tag="comb",
                                          name="comb")
                        v = ps36_sb.rearrange("t (k h c) -> t k h c", h=2, c=C)
                        combB = hpool.tile([NCELLS, BL * C], F32, tag="combB",
                                           name="combB")
                        nc.vector.tensor_tensor(
                            out=comb[:].rearrange("t (k c) -> t k c", c=C),
                            in0=v[0:NCELLS, :, 0, :], in1=v[0:NCELLS, :, 1, :],
                            op=mybir.AluOpType.add,
                        )
                        if ngc > 1:
                            nc.vector.tensor_tensor(
                                out=combB[:].rearrange("t (k c) -> t k c", c=C),
                                in0=v[64 : 64 + NCELLS, :, 0, :],
                                in1=v[64 : 64 + NCELLS, :, 1, :],
                                op=mybir.AluOpType.add,
                            )
                            nc.vector.tensor_tensor(
                                out=comb[:], in0=comb[:], in1=combB[:],
                                op=mybir.AluOpType.add,
                            )
                        for sl in range(BL):
                            nc.tensor.transpose(
                                out=pooledT_ps[gi][:, sl * NCELLS
                                                   : (sl + 1) * NCELLS],
                                in_=comb[0:NCELLS, sl * C : (sl + 1) * C],
                                identity=ident_sb[0:NCELLS, 0:NCELLS],
                            )
                        nc.vector.tensor_copy(out=pooled_sb[gi][:],
                                              in_=pooledT_ps[gi][:])
                else:
                    nc.vector.tensor_copy(out=pooled_sb[0][:], in_=pool_ps[0][:])
                    nc.vector.tensor_copy(out=pooled_sb[1][:], in_=pool_ps[1][:])

                emb_ps = ppool.tile([P, S], F32, tag="psA", space="PSUM", name="emb_ps")
                nc.tensor.matmul(out=emb_ps[:], lhsT=wemb_sb[:, 0:C],
                                 rhs=pooled_sb[0][:], start=True, stop=False)
                nc.tensor.matmul(out=emb_ps[:], lhsT=wemb_sb[:, C : 2 * C],
                                 rhs=pooled_sb[1][:], start=False, stop=True)
                embT_sb = hpool.tile([P, S], F32, tag="embT")
                nc.vector.tensor_scalar(
                    out=embT_sb[:], in0=emb_ps[:], scalar1=bemb_sb[:, 0:1],
                    scalar2=None, op0=mybir.AluOpType.add,
                )

                # ---- attention logits z = embT.T @ W_att + b_att -> [1, S] ----
                z_ps = ppool.tile([1, S], F32, tag="psB", space="PSUM", name="z_ps")
                nc.tensor.matmul(out=z_ps[:], lhsT=watt_sb[:], rhs=embT_sb[:],
                                 start=True, stop=True)
                z_sb = hpool.tile([1, S], F32, tag="z")
                nc.vector.tensor_scalar(
                    out=z_sb[:], in0=z_ps[:], scalar1=batt_sb[0:1, 0:1],
                    scalar2=None, op0=mybir.AluOpType.add,
                )

                # ---- per-graph softmax over the 36 cells ----
                att_sb = hpool.tile([1, S], F32, tag="att")
                mx_sb = hpool.tile([1, BL], F32, tag="mx")
                nmx_sb = hpool.tile([1, BL], F32, tag="nmx")
                ssum_sb = hpool.tile([1, BL], F32, tag="ssum")
                rcp_sb = hpool.tile([1, BL], F32, tag="rcp")
                for g in range(BL):
                    sl = slice(g * NCELLS, (g + 1) * NCELLS)
                    nc.vector.tensor_reduce(
                        out=mx_sb[0:1, g : g + 1], in_=z_sb[0:1, sl],
                        axis=mybir.AxisListType.X, op=mybir.AluOpType.max,
                    )
                    nc.vector.tensor_scalar(
                        out=nmx_sb[0:1, g : g + 1], in0=mx_sb[0:1, g : g + 1],
                        scalar1=-1.0, scalar2=None, op0=mybir.AluOpType.mult,
                    )
                    nc.scalar.activation(
                        out=att_sb[0:1, sl], in_=z_sb[0:1, sl],
                        func=mybir.ActivationFunctionType.Exp,
                        bias=nmx_sb[0:1, g : g + 1], scale=1.0,
                        accum_out=ssum_sb[0:1, g : g + 1],
                    )
                    nc.vector.reciprocal(rcp_sb[0:1, g : g + 1],
                                         ssum_sb[0:1, g : g + 1])
                    nc.vector.tensor_scalar(
                        out=att_sb[0:1, sl], in0=att_sb[0:1, sl],
                        scalar1=rcp_sb[0:1, g : g + 1], scalar2=None,
                        op0=mybir.AluOpType.mult,
                    )

                # ---- attended[H, G] = sum_cells att * embT ----
                attb_ps = ppool.tile([P, S], F32, tag="psC", space="PSUM", name="attb_ps")
                nc.tensor.matmul(out=attb_ps[:], lhsT=ones_sb[0:1, 0:P],
                                 rhs=att_sb[0:1, :], start=True, stop=True)
                wsum_sb = hpool.tile([P, S], F32, tag="wsum")
                nc.vector.tensor_tensor(
                    out=wsum_sb[:], in0=embT_sb[:], in1=attb_ps[:],
                    op=mybir.AluOpType.mult,
                )
                attd_sb = hpool.tile([P, BL], F32, tag="attd")
                for g in range(BL):
                    nc.vector.tensor_reduce(
                        out=attd_sb[:, g : g + 1],
                        in_=wsum_sb[:, g * NCELLS : (g + 1) * NCELLS],
                        axis=mybir.AxisListType.X, op=mybir.AluOpType.add,
                    )

                # ---- logits = attended.T @ W_fc + b_fc -> [G, OUT] ----
                log_ps = ppool.tile([BL, OUT], F32, tag="psD", space="PSUM", name="log_ps")
                nc.tensor.matmul(out=log_ps[:], lhsT=attd_sb[:, 0:BL],
                                 rhs=wfc_sb[:], start=True, stop=False)
                nc.tensor.matmul(out=log_ps[:], lhsT=ones_sb[0:1, 0:BL],
                                 rhs=bfc_sb[0:1, :], start=False, stop=True,
                                 skip_group_check=True)
                out_sb = hpool.tile([BL, OUT], F32, tag="out")
                nc.vector.tensor_copy(out=out_sb[:], in_=log_ps[:])
                nc.sync.dma_start(out=out[:], in_=out_sb[:])

            if repeat == 1:
                body()
            else:
                with tc.For_i(0, repeat, 1):
                    body()

    nc.compile()
    return nc


def _get(ng: int):
    if ng not in _cache:
        _cache[ng] = _build(ng)
    return _cache[ng]


def _patch_ids(pos: np.ndarray) -> np.ndarray:
    px = np.clip(np.floor(pos[:, 0] / np.float32(PATCH)), 0, NP_ - 1).astype(np.int32)
    py = np.clip(np.floor(pos[:, 1] / np.float32(PATCH)), 0, NP_ - 1).astype(np.int32)
    return px * NP_ + py


def prepare(x_graph_1, x_graph_2, pos_graph_1, pos_graph_2,
            batch_graph_1, batch_graph_2,
            W_emb, b_emb, W_att, b_att, W_fc, b_fc):
    """Compute per-core input maps + the slot size NG. Host-side only."""
    graphs = []
    for x, pos, batch in ((x_graph_1, pos_graph_1, batch_graph_1),
                          (x_graph_2, pos_graph_2, batch_graph_2)):
        x = np.asarray(x, dtype=np.float32)
        pos = np.asarray(pos, dtype=np.float32)
        batch = np.asarray(batch, dtype=np.int32)
        patch = _patch_ids(pos).astype(np.float32)
        bounds = np.searchsorted(batch, np.arange(B + 1)).astype(np.int64)
        if X_MODE == "hilo":
            xhi = x.astype(ml_dtypes.bfloat16)
            xlo = (x - xhi.astype(np.float32)).astype(ml_dtypes.bfloat16)
            x = (xhi, xlo)
        graphs.append((x, patch, bounds))

    span = max(int(np.diff(g[2]).max()) for g in graphs)
    ng = max(-(-span // P) * P, 2 * P)
    ngc = ng // P
    nchunks = BL * ngc
    nrows = BL * ng

    iota_tile = np.tile(np.arange(NCELLS, dtype=np.float32), (P, BUILD_K))
    ident_tile = np.eye(P, dtype=np.float32)
    w_emb = np.asarray(W_emb, dtype=np.float32)
    b_emb2 = np.asarray(b_emb, dtype=np.float32).reshape(H, 1)
    w_att = np.asarray(W_att, dtype=np.float32).reshape(H, 1)
    b_att2 = np.asarray(b_att, dtype=np.float32).reshape(1, 1)
    w_fc = np.asarray(W_fc, dtype=np.float32).reshape(H, OUT)
    b_fc2 = np.asarray(b_fc, dtype=np.float32).reshape(1, OUT)

    in_maps = []
    for k in range(M):
        im = {"iota": iota_tile, "ident": ident_tile, "wemb": w_emb,
              "bemb": b_emb2, "watt": w_att, "batt": b_att2, "wfc": w_fc,
              "bfc": b_fc2}
        for gi, (x, patch, bounds) in enumerate(graphs):
            if X_MODE == "hilo":
                xhi, xlo = x
                xbuf = np.zeros((nrows, 2 * C), dtype=ml_dtypes.bfloat16)
            else:
                xbuf = np.zeros((nrows, C), dtype=np.float32)
            segbuf = np.zeros(nrows, dtype=np.float32)
            for g in range(BL):
                lo = int(bounds[k * BL + g])
                hi = int(bounds[k * BL + g + 1])
                if X_MODE == "hilo":
                    xbuf[g * ng : g * ng + (hi - lo), 0:C] = xhi[lo:hi]
                    xbuf[g * ng : g * ng + (hi - lo), C : 2 * C] = xlo[lo:hi]
                else:
                    xbuf[g * ng : g * ng + (hi - lo)] = x[lo:hi]
                segbuf[g * ng : g * ng + (hi - lo)] = patch[lo:hi]
            im[f"x{gi + 1}"] = xbuf
            im[f"seg{gi + 1}"] = np.ascontiguousarray(
                segbuf.reshape(nchunks, P).T)
        in_maps.append(im)
    return ng, in_maps


def kernel(**inputs) -> np.ndarray:
    ng, in_maps = prepare(**inputs)
    nc = _get(ng)
    res = run_bass_kernel_spmd(nc, in_maps, list(range(M)))
    return np.concatenate([res.results[k]["out"] for k in range(M)], axis=0)


# revision 12
# speedup vs baseline: 1.2324x; 1.1832x over previous
"""Trainium2 Bass kernel for nn_Attention_Pooling_GNN (segment_reduce).

Strategy (data-parallel by graph, 8 cores):
  - Each core owns 4 of the 32 batch graphs. `batch` is sorted, so each
    graph's nodes are a contiguous row range of x. The host packs the 4
    graphs into 4 fixed-size row slots of NG rows each (zero rows pad the
    tail of a slot; zero rows add nothing to any segment), so the slot ->
    graph mapping is a compile-time constant shared by all 8 cores (SPMD).
  - On device, the segment sum is a one-hot matmul on the tensor engine:
    for every 128-node chunk, onehot[p, t] = (patch[p] == t) over only the
    36 spatial cells (the graph is known from the slot), built 8 chunks at
    a time by one vector-engine is_equal, then
    psum[C, 36*slot : 36*slot+36] += x_chunk.T @ onehot.
  - The tiny attention head (W_emb/W_att/W_fc + softmax over 36 cells) runs
    per-core on its own 4 graphs; the host concatenates the per-core [4, 2]
    logits into the final [32, 2].
"""

import ml_dtypes
import numpy as np

from concourse import bacc, mybir
from concourse.tile import TileContext
from concourse.bass_utils import run_bass_kernel_spmd

# Problem constants (must match reference.setup_inputs()).
N = 200000
C = 128
H = 128
OUT = 2
B = 32
GRID = 1216
PATCH = 202
NP_ = 6             # patches per axis
NCELLS = NP_ * NP_  # 36
M = 8               # cores
BL = B // M         # graphs per core = 4
S = NCELLS * BL     # 144 segments per core
P = 128

SLAB = 16           # chunks per x DMA (1 MiB)
BUILD_K = 8         # chunks whose one-hots are built by one vector op

F32 = mybir.dt.float32
BF16 = mybir.dt.bfloat16

# x representation: "f32" exact (slow fp32 matmuls), "bf16" (cast during
# DMA, ~8e-3 rel err), or "hilo" (host splits x into bf16 hi+lo halves --
# same HBM bytes as fp32, ~1e-5 rel err, bf16-speed matmuls)
X_MODE = "hilo"

# ablation for profiling: "" full, "dma" = DMAs only, "nodve" = no one-hot
# builds (matmuls read a stale tile), "nomm" = no matmuls
import os
ABLATE = os.environ.get("KERNEL_ABLATE", "")

_cache: dict = {}


def _build(ng: int, repeat: int = 1):
    """Build the per-core program. `repeat` > 1 wraps the whole body in an
    on-device loop - used only for timing (amortizes launch overhead)."""
    ngc = ng // P            # chunks per graph slot
    nchunks = BL * ngc       # chunks per graph input
    nrows = BL * ng
    nc = bacc.Bacc("TRN2", target_bir_lowering=False, debug=False, num_devices=M)

    xw = 2 * C if X_MODE == "hilo" else C
    xdrdt = BF16 if X_MODE == "hilo" else F32
    x1 = nc.declare_dram_parameter("x1", [nrows, xw], xdrdt, isOutput=False)
    x2 = nc.declare_dram_parameter("x2", [nrows, xw], xdrdt, isOutput=False)
    seg1 = nc.declare_dram_parameter("seg1", [P, nchunks], F32, isOutput=False)
    seg2 = nc.declare_dram_parameter("seg2", [P, nchunks], F32, isOutput=False)
    iota_in = nc.declare_dram_parameter("iota", [P, BUILD_K * NCELLS], F32,
                                        isOutput=False)
    wemb = nc.declare_dram_parameter("wemb", [2 * C, H], F32, isOutput=False)
    bemb = nc.declare_dram_parameter("bemb", [H, 1], F32, isOutput=False)
    watt = nc.declare_dram_parameter("watt", [H, 1], F32, isOutput=False)
    batt = nc.declare_dram_parameter("batt", [1, 1], F32, isOutput=False)
    wfc = nc.declare_dram_parameter("wfc", [H, OUT], F32, isOutput=False)
    bfc = nc.declare_dram_parameter("bfc", [1, OUT], F32, isOutput=False)
    ident = nc.declare_dram_parameter("ident", [P, P], F32, isOutput=False)
    out = nc.declare_dram_parameter("out", [BL, OUT], F32, isOutput=True)

    with TileContext(nc) as tc:
        with (
            tc.tile_pool(name="const", bufs=1) as cpool,
            tc.tile_pool(name="xs", bufs=3) as xpool,
            tc.tile_pool(name="oh", bufs=4) as ohpool,
            tc.tile_pool(name="head", bufs=1) as hpool,
            tc.tile_pool(name="psum", bufs=1, space="PSUM") as ppool,
        ):
            iota_sb = cpool.tile([P, BUILD_K * NCELLS], F32, tag="iota")
            nc.sync.dma_start(out=iota_sb[:], in_=iota_in[:])
            seg_sb = [cpool.tile([P, nchunks], F32, tag=f"seg{g}",
                                 name=f"seg{g}_sb") for g in (1, 2)]
            nc.sync.dma_start(out=seg_sb[0][:], in_=seg1[:])
            nc.sync.dma_start(out=seg_sb[1][:], in_=seg2[:])

            wemb_sb = cpool.tile([P, 2 * C], F32, tag="wemb")
            nc.sync.dma_start(out=wemb_sb[:, 0:C], in_=wemb[0:C, :])
            nc.sync.dma_start(out=wemb_sb[:, C : 2 * C], in_=wemb[C : 2 * C, :])
            bemb_sb = cpool.tile([P, 1], F32, tag="bemb")
            nc.sync.dma_start(out=bemb_sb[:], in_=bemb[:])
            watt_sb = cpool.tile([P, 1], F32, tag="watt")
            nc.sync.dma_start(out=watt_sb[:], in_=watt[:])
            batt_sb = cpool.tile([1, 1], F32, tag="batt")
            nc.sync.dma_start(out=batt_sb[:], in_=batt[:])
            wfc_sb = cpool.tile([P, OUT], F32, tag="wfc")
            nc.sync.dma_start(out=wfc_sb[:], in_=wfc[:])
            bfc_sb = cpool.tile([1, OUT], F32, tag="bfc")
            nc.sync.dma_start(out=bfc_sb[:], in_=bfc[:])
            ones_sb = cpool.tile([1, P], F32, tag="ones")
            nc.vector.memset(ones_sb[:], 1.0)
            ident_sb = cpool.tile([P, P], F32, tag="ident")
            nc.sync.dma_start(out=ident_sb[:], in_=ident[:])

            def body():
                if X_MODE == "hilo":
                    # [cell-band, slot*(hi|lo)*C] partial sums: even chunks
                    # accumulate into partitions 0:36 (PE col-group 0), odd
                    # chunks into partitions 64:100 (col-group 64) so pairs of
                    # matmuls run concurrently; combined + transposed into
                    # [C, S] after the loop
                    pool_ps = [
                        ppool.tile([P, BL * 2 * C], F32, tag=t,
                                   space="PSUM", name=f"pool_ps{t}")
                        for t in ("psA", "psB")
                    ]
                else:
                    pool_ps = [
                        ppool.tile([P, S], F32, tag=t, space="PSUM",
                                   name=f"pool_ps{t}")
                        for t in ("psA", "psB")
                    ]

                # ---- segment-sum ----
                for gi, xin in enumerate((x1, x2)):
                    xv = xin[:].rearrange("(n p) c -> p n c", p=P)
                    ps = pool_ps[gi]
                    oh = None
                    for s0 in range(0, nchunks, SLAB):
                        cnt = min(SLAB, nchunks - s0)
                        xdt = F32 if X_MODE == "f32" else BF16
                        xt = xpool.tile([P, SLAB * xw], xdt, tag="xslab")
                        dma_eng = nc.gpsimd if X_MODE == "bf16" else nc.sync
                        dma_eng.dma_start(
                            out=xt[:, : cnt * xw].rearrange("p (n c) -> p n c", c=xw),
                            in_=xv[:, s0 : s0 + cnt, :],
                        )
                        for j in range(cnt):
                            ci = s0 + j
                            bj = ci % BUILD_K
                            if bj == 0 and ABLATE not in ("dma", "nodve"):
                                # one is_equal builds one-hots for BUILD_K
                                # chunks: seg col broadcast over the 36 cells
                                bk = min(BUILD_K, nchunks - ci)
                                oh = ohpool.tile(
                                    [P, BUILD_K * NCELLS],
                                    F32 if X_MODE == "f32" else BF16,
                                    tag="oh", name="oh")
                                nc.vector.tensor_tensor(
                                    out=oh[:, : bk * NCELLS].rearrange(
                                        "p (k t) -> p k t", t=NCELLS),
                                    in0=iota_sb[:, : bk * NCELLS].rearrange(
                                        "p (k t) -> p k t", t=NCELLS),
                                    in1=seg_sb[gi][:, ci : ci + bk, None]
                                        .to_broadcast([P, bk, NCELLS]),
                                    op=mybir.AluOpType.is_equal,
                                )
                            slot = ci // ngc
                            loc = ci % ngc
                            if ABLATE in ("dma", "nomm"):
                                continue
                            if oh is None:
                                oh = ohpool.tile(
                                    [P, BUILD_K * NCELLS],
                                    F32 if X_MODE == "f32" else BF16,
                                    tag="oh", name="oh")
                                nc.vector.memset(oh[:], 0.0)
                            if X_MODE == "hilo":
                                # out[cell, (hi|lo)*C] += onehot.T @ [xhi|xlo]
                                band = 64 * (loc % 2)
                                last = ngc - 1 - ((ngc - 1 - loc % 2) % 2)
                                nc.tensor.matmul(
                                    out=ps[band : band + NCELLS,
                                           slot * 2 * C : (slot + 1) * 2 * C],
                                    lhsT=oh[:, bj * NCELLS : (bj + 1) * NCELLS],
                                    rhs=xt[:, j * xw : (j + 1) * xw],
                                    start=(loc == loc % 2),
                                    stop=(loc == last),
                                    tile_position=(0, band),
                                )
                            else:
                                nc.tensor.matmul(
                                    out=ps[:, slot * NCELLS : (slot + 1) * NCELLS],
                                    lhsT=xt[:, j * C : (j + 1) * C],
                                    rhs=oh[:, bj * NCELLS : (bj + 1) * NCELLS],
                                    start=(loc == 0),
                                    stop=(loc == ngc - 1),
                                )

                # ---- head: embT[H, S] = W_emb.T @ pooled (+ b_emb) ----
                pooled_sb = [hpool.tile([P, S], F32, tag=f"pooled{g}",
                                        name=f"pooled{g}_sb") for g in (1, 2)]
                if ABLATE in ("dma", "nomm"):
                    for t in pool_ps:
                        nc.vector.memset(t[:, 0:S], 0.0)
                if X_MODE == "hilo" and ABLATE in ("dma", "nomm"):
                    for gi in range(2):
                        nc.vector.memset(pooled_sb[gi][:], 0.0)
                elif X_MODE == "hilo":
                    # hi+lo recombine, then PE-transpose [cell, C] -> [C, cell]
                    pooledT_ps = [
                        ppool.tile([P, S], F32, tag=t, space="PSUM",
                                   name=f"pooledT_{t}")
                        for t in ("psC", "psD")
                    ]
                    for gi in range(2):
                        ps36_sb = hpool.tile([P, BL * 2 * C], F32,
                                             tag="ps36_sb", name="ps36_sb")
                        nc.vector.tensor_copy(out=ps36_sb[:],
                                              in_=pool_ps[gi][:])
                        comb = hpool.tile([NCELLS, BL * C], F32, tag="comb",
                                          name="comb")
                        v = ps36_sb.rearrange("t (k h c) -> t k h c", h=2, c=C)
                        combB = hpool.tile([NCELLS, BL * C], F32, tag="combB",
                                           name="combB")
                        nc.vector.tensor_tensor(
                            out=comb[:].rearrange("t (k c) -> t k c", c=C),
                            in0=v[0:NCELLS, :, 0, :], in1=v[0:NCELLS, :, 1, :],
                            op=mybir.AluOpType.add,
                        )
                        if ngc > 1:
                            nc.vector.tensor_tensor(
                                out=combB[:].rearrange("t (k c) -> t k c", c=C),
                                in0=v[64 : 64 + NCELLS, :, 0, :],
                                in1=v[64 : 64 + NCELLS, :, 1, :],
                                op=mybir.AluOpType.add,
                            )
                            nc.vector.tensor_tensor(
                                out=comb[:], in0=comb[:], in1=combB[:],
                                op=mybir.AluOpType.add,
                            )
                        for sl in range(BL):
                            nc.tensor.transpose(
                                out=pooledT_ps[gi][:, sl * NCELLS
                                                   : (sl + 1) * NCELLS],
                                in_=comb[0:NCELLS, sl * C : (sl + 1) * C],
                                identity=ident_sb[0:NCELLS, 0:NCELLS],
                            )
                        nc.vector.tensor_copy(out=pooled_sb[gi][:],
                                              in_=pooledT_ps[gi][:])
                else:
                    nc.vector.tensor_copy(out=pooled_sb[0][:], in_=pool_ps[0][:])
                    nc.vector.tensor_copy(out=pooled_sb[1][:], in_=pool_ps[1][:])

                emb_ps = ppool.tile([P, S], F32, tag="psA", space="PSUM", name="emb_ps")
                nc.tensor.matmul(out=emb_ps[:], lhsT=wemb_sb[:, 0:C],
                                 rhs=pooled_sb[0][:], start=True, stop=False)
                nc.tensor.matmul(out=emb_ps[:], lhsT=wemb_sb[:, C : 2 * C],
                                 rhs=pooled_sb[1][:], start=False, stop=True)
                embT_sb = hpool.tile([P, S], F32, tag="embT")
                nc.vector.tensor_scalar(
                    out=embT_sb[:], in0=emb_ps[:], scalar1=bemb_sb[:, 0:1],
                    scalar2=None, op0=mybir.AluOpType.add,
                )

                # ---- attention logits z = embT.T @ W_att + b_att -> [1, S] ----
                z_ps = ppool.tile([1, S], F32, tag="psB", space="PSUM", name="z_ps")
                nc.tensor.matmul(out=z_ps[:], lhsT=watt_sb[:], rhs=embT_sb[:],
                                 start=True, stop=True)
                z_sb = hpool.tile([1, S], F32, tag="z")
                nc.vector.tensor_scalar(
                    out=z_sb[:], in0=z_ps[:], scalar1=batt_sb[0:1, 0:1],
                    scalar2=None, op0=mybir.AluOpType.add,
                )

                # ---- per-graph softmax over the 36 cells ----
                att_sb = hpool.tile([1, S], F32, tag="att")
                mx_sb = hpool.tile([1, BL], F32, tag="mx")
                nmx_sb = hpool.tile([1, BL], F32, tag="nmx")
                ssum_sb = hpool.tile([1, BL], F32, tag="ssum")
                rcp_sb = hpool.tile([1, BL], F32, tag="rcp")
                for g in range(BL):
                    sl = slice(g * NCELLS, (g + 1) * NCELLS)
                    nc.vector.tensor_reduce(
                        out=mx_sb[0:1, g : g + 1], in_=z_sb[0:1, sl],
                        axis=mybir.AxisListType.X, op=mybir.AluOpType.max,
                    )
                    nc.vector.tensor_scalar(
                        out=nmx_sb[0:1, g : g + 1], in0=mx_sb[0:1, g : g + 1],
                        scalar1=-1.0, scalar2=None, op0=mybir.AluOpType.mult,
                    )
                    nc.scalar.activation(
                        out=att_sb[0:1, sl], in_=z_sb[0:1, sl],
                        func=mybir.ActivationFunctionType.Exp,
                        bias=nmx_sb[0:1, g : g + 1], scale=1.0,
                        accum_out=ssum_sb[0:1, g : g + 1],
                    )
                    nc.vector.reciprocal(rcp_sb[0:1, g : g + 1],
                                         ssum_sb[0:1, g : g + 1])
                    nc.vector.tensor_scalar(
                        out=att_sb[0:1, sl], in0=att_sb[0:1, sl],
                        scalar1=rcp_sb[0:1, g : g + 1], scalar2=None,
                        op0=mybir.AluOpType.mult,
                    )

                # ---- attended[H, G] = sum_cells att * embT ----
                attb_ps = ppool.tile([P, S], F32, tag="psC", space="PSUM", name="attb_ps")
                nc.tensor.matmul(out=attb_ps[:], lhsT=ones_sb[0:1, 0:P],
                                 rhs=att_sb[0:1, :], start=True, stop=True)
                wsum_sb = hpool.tile([P, S], F32, tag="wsum")
                nc.vector.tensor_tensor(
                    out=wsum_sb[:], in0=embT_sb[:], in1=attb_ps[:],
                    op=mybir.AluOpType.mult,
                )
                attd_sb = hpool.tile([P, BL], F32, tag="attd")
                for g in range(BL):
                    nc.vector.tensor_reduce(
                        out=attd_sb[:, g : g + 1],
                        in_=wsum_sb[:, g * NCELLS : (g + 1) * NCELLS],
                        axis=mybir.AxisListType.X, op=mybir.AluOpType.add,
                    )

                # ---- logits = attended.T @ W_fc + b_fc -> [G, OUT] ----
                log_ps = ppool.tile([BL, OUT], F32, tag="psD", space="PSUM", name="log_ps")
                nc.tensor.matmul(out=log_ps[:], lhsT=attd_sb[:, 0:BL],
                                 rhs=wfc_sb[:], start=True, stop=False)
                nc.tensor.matmul(out=log_ps[:], lhsT=ones_sb[0:1, 0:BL],
                                 rhs=bfc_sb[0:1, :], start=False, stop=True,
                                 skip_group_check=True)
                out_sb = hpool.tile([BL, OUT], F32, tag="out")
                nc.vector.tensor_copy(out=out_sb[:], in_=log_ps[:])
                nc.sync.dma_start(out=out[:], in_=out_sb[:])

            if repeat == 1:
                body()
            else:
                with tc.For_i(0, repeat, 1):
                    body()

    nc.compile()
    return nc


def _get(ng: int):
    if ng not in _cache:
        _cache[ng] = _build(ng)
    return _cache[ng]


def _patch_ids(pos: np.ndarray) -> np.ndarray:
    px = np.clip(np.floor(pos[:, 0] / np.float32(PATCH)), 0, NP_ - 1).astype(np.int32)
    py = np.clip(np.floor(pos[:, 1] / np.float32(PATCH)), 0, NP_ - 1).astype(np.int32)
    return px * NP_ + py


def prepare(x_graph_1, x_graph_2, pos_graph_1, pos_graph_2,
            batch_graph_1, batch_graph_2,
            W_emb, b_emb, W_att, b_att, W_fc, b_fc):
    """Compute per-core input maps + the slot size NG. Host-side only."""
    graphs = []
    for x, pos, batch in ((x_graph_1, pos_graph_1, batch_graph_1),
                          (x_graph_2, pos_graph_2, batch_graph_2)):
        x = np.asarray(x, dtype=np.float32)
        pos = np.asarray(pos, dtype=np.float32)
        batch = np.asarray(batch, dtype=np.int32)
        patch = _patch_ids(pos).astype(np.float32)
        bounds = np.searchsorted(batch, np.arange(B + 1)).astype(np.int64)
        if X_MODE == "hilo":
            xhi = x.astype(ml_dtypes.bfloat16)
            xlo = (x - xhi.astype(np.float32)).astype(ml_dtypes.bfloat16)
            x = (xhi, xlo)
        graphs.append((x, patch, bounds))

    span = max(int(np.diff(g[2]).max()) for g in graphs)
    ng = max(-(-span // P) * P, 2 * P)
    ngc = ng // P
    nchunks = BL * ngc
    nrows = BL * ng

    iota_tile = np.tile(np.arange(NCELLS, dtype=np.float32), (P, BUILD_K))
    ident_tile = np.eye(P, dtype=np.float32)
    w_emb = np.asarray(W_emb, dtype=np.float32)
    b_emb2 = np.asarray(b_emb, dtype=np.float32).reshape(H, 1)
    w_att = np.asarray(W_att, dtype=np.float32).reshape(H, 1)
    b_att2 = np.asarray(b_att, dtype=np.float32).reshape(1, 1)
    w_fc = np.asarray(W_fc, dtype=np.float32).reshape(H, OUT)
    b_fc2 = np.asarray(b_fc, dtype=np.float32).reshape(1, OUT)

    in_maps = []
    for k in range(M):
        im = {"iota": iota_tile, "ident": ident_tile, "wemb": w_emb,
              "bemb": b_emb2, "watt": w_att, "batt": b_att2, "wfc": w_fc,
              "bfc": b_fc2}
        for gi, (x, patch, bounds) in enumerate(graphs):
            if X_MODE == "hilo":
                xhi, xlo = x
                xbuf = np.zeros((nrows, 2 * C), dtype=ml_dtypes.bfloat16)
            else:
                xbuf = np.zeros((nrows, C), dtype=np.float32)
            segbuf = np.zeros(nrows, dtype=np.float32)
            for g in range(BL):
                lo = int(bounds[k * BL + g])
                hi = int(bounds[k * BL + g + 1])
                if X_MODE == "hilo":
                    xbuf[g * ng : g * ng + (hi - lo), 0:C] = xhi[lo:hi]
                    xbuf[g * ng : g * ng + (hi - lo), C : 2 * C] = xlo[lo:hi]
                else:
                    xbuf[g * ng : g * ng + (hi - lo)] = x[lo:hi]
                segbuf[g * ng : g * ng + (hi - lo)] = patch[lo:hi]
            im[f"x{gi + 1}"] = xbuf
            im[f"seg{gi + 1}"] = np.ascontiguousarray(
                segbuf.reshape(nchunks, P).T)
        in_maps.append(im)
    return ng, in_maps


def kernel(**inputs) -> np.ndarray:
    ng, in_maps = prepare(**inputs)
    nc = _get(ng)
    res = run_bass_kernel_spmd(nc, in_maps, list(range(M)))
    return np.concatenate([res.results[k]["out"] for k in range(M)], axis=0)


# revision 13
# speedup vs baseline: 1.5524x; 1.2597x over previous
"""Trainium2 Bass kernel for nn_Attention_Pooling_GNN (segment_reduce).

Strategy (data-parallel by graph, 8 cores):
  - Each core owns 4 of the 32 batch graphs. `batch` is sorted, so each
    graph's nodes are a contiguous row range of x. The host packs the 4
    graphs into 4 fixed-size row slots of NG rows each (zero rows pad the
    tail of a slot; zero rows add nothing to any segment), so the slot ->
    graph mapping is a compile-time constant shared by all 8 cores (SPMD).
  - On device, the segment sum is a one-hot matmul on the tensor engine:
    for every 128-node chunk, onehot[p, t] = (patch[p] == t) over only the
    36 spatial cells (the graph is known from the slot), built 8 chunks at
    a time by one vector-engine is_equal, then
    psum[C, 36*slot : 36*slot+36] += x_chunk.T @ onehot.
  - The tiny attention head (W_emb/W_att/W_fc + softmax over 36 cells) runs
    per-core on its own 4 graphs; the host concatenates the per-core [4, 2]
    logits into the final [32, 2].
"""

import ml_dtypes
import numpy as np

from concourse import bacc, mybir
from concourse.tile import TileContext
from concourse.bass_utils import run_bass_kernel_spmd

# Problem constants (must match reference.setup_inputs()).
N = 200000
C = 128
H = 128
OUT = 2
B = 32
GRID = 1216
PATCH = 202
NP_ = 6             # patches per axis
NCELLS = NP_ * NP_  # 36
M = 8               # cores
BL = B // M         # graphs per core = 4
S = NCELLS * BL     # 144 segments per core
P = 128

SLAB = 16           # chunks per x DMA (1 MiB)
BUILD_K = 8         # chunks whose one-hots are built by one vector op

F32 = mybir.dt.float32
BF16 = mybir.dt.bfloat16

# x representation: "f32" exact (slow fp32 matmuls), "bf16" (cast during
# DMA, ~8e-3 rel err), or "hilo" (host splits x into bf16 hi+lo halves --
# same HBM bytes as fp32, ~1e-5 rel err, bf16-speed matmuls)
X_MODE = "hilo"

# ablation for profiling: "" full, "dma" = DMAs only, "nodve" = no one-hot
# builds (matmuls read a stale tile), "nomm" = no matmuls
import os
ABLATE = os.environ.get("KERNEL_ABLATE", "")

_cache: dict = {}


def _build(ng: int, repeat: int = 1):
    """Build the per-core program. `repeat` > 1 wraps the whole body in an
    on-device loop - used only for timing (amortizes launch overhead)."""
    ngc = ng // P            # chunks per graph slot
    nchunks = BL * ngc       # chunks per graph input
    nrows = BL * ng
    nc = bacc.Bacc("TRN2", target_bir_lowering=False, debug=False, num_devices=M)

    xw = 2 * C if X_MODE == "hilo" else C
    xdrdt = BF16 if X_MODE == "hilo" else F32
    # partition-major layout: x[p, n*xw + c] = row (n*128+p), col c -- each
    # partition's slab bytes are contiguous in HBM (one fat DMA descriptor)
    x1 = nc.declare_dram_parameter("x1", [P, nchunks * xw], xdrdt, isOutput=False)
    x2 = nc.declare_dram_parameter("x2", [P, nchunks * xw], xdrdt, isOutput=False)
    seg1 = nc.declare_dram_parameter("seg1", [P, nchunks], F32, isOutput=False)
    seg2 = nc.declare_dram_parameter("seg2", [P, nchunks], F32, isOutput=False)
    iota_in = nc.declare_dram_parameter("iota", [P, BUILD_K * NCELLS], F32,
                                        isOutput=False)
    wemb = nc.declare_dram_parameter("wemb", [2 * C, H], F32, isOutput=False)
    bemb = nc.declare_dram_parameter("bemb", [H, 1], F32, isOutput=False)
    watt = nc.declare_dram_parameter("watt", [H, 1], F32, isOutput=False)
    batt = nc.declare_dram_parameter("batt", [1, 1], F32, isOutput=False)
    wfc = nc.declare_dram_parameter("wfc", [H, OUT], F32, isOutput=False)
    bfc = nc.declare_dram_parameter("bfc", [1, OUT], F32, isOutput=False)
    ident = nc.declare_dram_parameter("ident", [P, P], F32, isOutput=False)
    out = nc.declare_dram_parameter("out", [BL, OUT], F32, isOutput=True)

    with TileContext(nc) as tc:
        with (
            tc.tile_pool(name="const", bufs=1) as cpool,
            tc.tile_pool(name="xs", bufs=3) as xpool,
            tc.tile_pool(name="oh", bufs=4) as ohpool,
            tc.tile_pool(name="head", bufs=1) as hpool,
            tc.tile_pool(name="psum", bufs=1, space="PSUM") as ppool,
        ):
            iota_sb = cpool.tile([P, BUILD_K * NCELLS], F32, tag="iota")
            nc.sync.dma_start(out=iota_sb[:], in_=iota_in[:])
            seg_sb = [cpool.tile([P, nchunks], F32, tag=f"seg{g}",
                                 name=f"seg{g}_sb") for g in (1, 2)]
            nc.sync.dma_start(out=seg_sb[0][:], in_=seg1[:])
            nc.sync.dma_start(out=seg_sb[1][:], in_=seg2[:])

            wemb_sb = cpool.tile([P, 2 * C], F32, tag="wemb")
            nc.sync.dma_start(out=wemb_sb[:, 0:C], in_=wemb[0:C, :])
            nc.sync.dma_start(out=wemb_sb[:, C : 2 * C], in_=wemb[C : 2 * C, :])
            bemb_sb = cpool.tile([P, 1], F32, tag="bemb")
            nc.sync.dma_start(out=bemb_sb[:], in_=bemb[:])
            watt_sb = cpool.tile([P, 1], F32, tag="watt")
            nc.sync.dma_start(out=watt_sb[:], in_=watt[:])
            batt_sb = cpool.tile([1, 1], F32, tag="batt")
            nc.sync.dma_start(out=batt_sb[:], in_=batt[:])
            wfc_sb = cpool.tile([P, OUT], F32, tag="wfc")
            nc.sync.dma_start(out=wfc_sb[:], in_=wfc[:])
            bfc_sb = cpool.tile([1, OUT], F32, tag="bfc")
            nc.sync.dma_start(out=bfc_sb[:], in_=bfc[:])
            ones_sb = cpool.tile([1, P], F32, tag="ones")
            nc.vector.memset(ones_sb[:], 1.0)
            ident_sb = cpool.tile([P, P], F32, tag="ident")
            nc.sync.dma_start(out=ident_sb[:], in_=ident[:])

            def body():
                if X_MODE == "hilo":
                    # [cell-band, slot*(hi|lo)*C] partial sums: even chunks
                    # accumulate into partitions 0:36 (PE col-group 0), odd
                    # chunks into partitions 64:100 (col-group 64) so pairs of
                    # matmuls run concurrently; combined + transposed into
                    # [C, S] after the loop
                    pool_ps = [
                        ppool.tile([P, BL * 2 * C], F32, tag=t,
                                   space="PSUM", name=f"pool_ps{t}")
                        for t in ("psA", "psB")
                    ]
                else:
                    pool_ps = [
                        ppool.tile([P, S], F32, tag=t, space="PSUM",
                                   name=f"pool_ps{t}")
                        for t in ("psA", "psB")
                    ]

                # ---- segment-sum ----
                for gi, xin in enumerate((x1, x2)):
                    ps = pool_ps[gi]
                    oh = None
                    for s0 in range(0, nchunks, SLAB):
                        cnt = min(SLAB, nchunks - s0)
                        xdt = F32 if X_MODE == "f32" else BF16
                        xt = xpool.tile([P, SLAB * xw], xdt, tag="xslab")
                        dma_eng = nc.gpsimd if X_MODE == "bf16" else nc.sync
                        dma_eng.dma_start(
                            out=xt[:, : cnt * xw],
                            in_=xin[:, s0 * xw : (s0 + cnt) * xw],
                        )
                        for j in range(cnt):
                            ci = s0 + j
                            bj = ci % BUILD_K
                            if bj == 0 and ABLATE not in ("dma", "nodve"):
                                # one is_equal builds one-hots for BUILD_K
                                # chunks: seg col broadcast over the 36 cells
                                bk = min(BUILD_K, nchunks - ci)
                                oh = ohpool.tile(
                                    [P, BUILD_K * NCELLS],
                                    F32 if X_MODE == "f32" else BF16,
                                    tag="oh", name="oh")
                                nc.vector.tensor_tensor(
                                    out=oh[:, : bk * NCELLS].rearrange(
                                        "p (k t) -> p k t", t=NCELLS),
                                    in0=iota_sb[:, : bk * NCELLS].rearrange(
                                        "p (k t) -> p k t", t=NCELLS),
                                    in1=seg_sb[gi][:, ci : ci + bk, None]
                                        .to_broadcast([P, bk, NCELLS]),
                                    op=mybir.AluOpType.is_equal,
                                )
                            slot = ci // ngc
                            loc = ci % ngc
                            if ABLATE in ("dma", "nomm"):
                                continue
                            if oh is None:
                                oh = ohpool.tile(
                                    [P, BUILD_K * NCELLS],
                                    F32 if X_MODE == "f32" else BF16,
                                    tag="oh", name="oh")
                                nc.vector.memset(oh[:], 0.0)
                            if X_MODE == "hilo":
                                # out[cell, (hi|lo)*C] += onehot.T @ [xhi|xlo]
                                band = 64 * (loc % 2)
                                last = ngc - 1 - ((ngc - 1 - loc % 2) % 2)
                                nc.tensor.matmul(
                                    out=ps[band : band + NCELLS,
                                           slot * 2 * C : (slot + 1) * 2 * C],
                                    lhsT=oh[:, bj * NCELLS : (bj + 1) * NCELLS],
                                    rhs=xt[:, j * xw : (j + 1) * xw],
                                    start=(loc == loc % 2),
                                    stop=(loc == last),
                                    tile_position=(0, band),
                                )
                            else:
                                nc.tensor.matmul(
                                    out=ps[:, slot * NCELLS : (slot + 1) * NCELLS],
                                    lhsT=xt[:, j * C : (j + 1) * C],
                                    rhs=oh[:, bj * NCELLS : (bj + 1) * NCELLS],
                                    start=(loc == 0),
                                    stop=(loc == ngc - 1),
                                )

                # ---- head: embT[H, S] = W_emb.T @ pooled (+ b_emb) ----
                pooled_sb = [hpool.tile([P, S], F32, tag=f"pooled{g}",
                                        name=f"pooled{g}_sb") for g in (1, 2)]
                if ABLATE in ("dma", "nomm"):
                    for t in pool_ps:
                        nc.vector.memset(t[:, 0:S], 0.0)
                if X_MODE == "hilo" and ABLATE in ("dma", "nomm"):
                    for gi in range(2):
                        nc.vector.memset(pooled_sb[gi][:], 0.0)
                elif X_MODE == "hilo":
                    # hi+lo recombine, then PE-transpose [cell, C] -> [C, cell]
                    pooledT_ps = [
                        ppool.tile([P, S], F32, tag=t, space="PSUM",
                                   name=f"pooledT_{t}")
                        for t in ("psC", "psD")
                    ]
                    for gi in range(2):
                        ps36_sb = hpool.tile([P, BL * 2 * C], F32,
                                             tag="ps36_sb", name="ps36_sb")
                        nc.vector.tensor_copy(out=ps36_sb[:],
                                              in_=pool_ps[gi][:])
                        comb = hpool.tile([NCELLS, BL * C], F32, tag="comb",
                                          name="comb")
                        v = ps36_sb.rearrange("t (k h c) -> t k h c", h=2, c=C)
                        combB = hpool.tile([NCELLS, BL * C], F32, tag="combB",
                                           name="combB")
                        nc.vector.tensor_tensor(
                            out=comb[:].rearrange("t (k c) -> t k c", c=C),
                            in0=v[0:NCELLS, :, 0, :], in1=v[0:NCELLS, :, 1, :],
                            op=mybir.AluOpType.add,
                        )
                        if ngc > 1:
                            nc.vector.tensor_tensor(
                                out=combB[:].rearrange("t (k c) -> t k c", c=C),
                                in0=v[64 : 64 + NCELLS, :, 0, :],
                                in1=v[64 : 64 + NCELLS, :, 1, :],
                                op=mybir.AluOpType.add,
                            )
                            nc.vector.tensor_tensor(
                                out=comb[:], in0=comb[:], in1=combB[:],
                                op=mybir.AluOpType.add,
                            )
                        for sl in range(BL):
                            nc.tensor.transpose(
                                out=pooledT_ps[gi][:, sl * NCELLS
                                                   : (sl + 1) * NCELLS],
                                in_=comb[0:NCELLS, sl * C : (sl + 1) * C],
                                identity=ident_sb[0:NCELLS, 0:NCELLS],
                            )
                        nc.vector.tensor_copy(out=pooled_sb[gi][:],
                                              in_=pooledT_ps[gi][:])
                else:
                    nc.vector.tensor_copy(out=pooled_sb[0][:], in_=pool_ps[0][:])
                    nc.vector.tensor_copy(out=pooled_sb[1][:], in_=pool_ps[1][:])

                emb_ps = ppool.tile([P, S], F32, tag="psA", space="PSUM", name="emb_ps")
                nc.tensor.matmul(out=emb_ps[:], lhsT=wemb_sb[:, 0:C],
                                 rhs=pooled_sb[0][:], start=True, stop=False)
                nc.tensor.matmul(out=emb_ps[:], lhsT=wemb_sb[:, C : 2 * C],
                                 rhs=pooled_sb[1][:], start=False, stop=True)
                embT_sb = hpool.tile([P, S], F32, tag="embT")
                nc.vector.tensor_scalar(
                    out=embT_sb[:], in0=emb_ps[:], scalar1=bemb_sb[:, 0:1],
                    scalar2=None, op0=mybir.AluOpType.add,
                )

                # ---- attention logits z = embT.T @ W_att + b_att -> [1, S] ----
                z_ps = ppool.tile([1, S], F32, tag="psB", space="PSUM", name="z_ps")
                nc.tensor.matmul(out=z_ps[:], lhsT=watt_sb[:], rhs=embT_sb[:],
                                 start=True, stop=True)
                z_sb = hpool.tile([1, S], F32, tag="z")
                nc.vector.tensor_scalar(
                    out=z_sb[:], in0=z_ps[:], scalar1=batt_sb[0:1, 0:1],
                    scalar2=None, op0=mybir.AluOpType.add,
                )

                # ---- per-graph softmax over the 36 cells ----
                att_sb = hpool.tile([1, S], F32, tag="att")
                mx_sb = hpool.tile([1, BL], F32, tag="mx")
                nmx_sb = hpool.tile([1, BL], F32, tag="nmx")
                ssum_sb = hpool.tile([1, BL], F32, tag="ssum")
                rcp_sb = hpool.tile([1, BL], F32, tag="rcp")
                for g in range(BL):
                    sl = slice(g * NCELLS, (g + 1) * NCELLS)
                    nc.vector.tensor_reduce(
                        out=mx_sb[0:1, g : g + 1], in_=z_sb[0:1, sl],
                        axis=mybir.AxisListType.X, op=mybir.AluOpType.max,
                    )
                    nc.vector.tensor_scalar(
                        out=nmx_sb[0:1, g : g + 1], in0=mx_sb[0:1, g : g + 1],
                        scalar1=-1.0, scalar2=None, op0=mybir.AluOpType.mult,
                    )
                    nc.scalar.activation(
                        out=att_sb[0:1, sl], in_=z_sb[0:1, sl],
                        func=mybir.ActivationFunctionType.Exp,
                        bias=nmx_sb[0:1, g : g + 1], scale=1.0,
                        accum_out=ssum_sb[0:1, g : g + 1],
                    )
                    nc.vector.reciprocal(rcp_sb[0:1, g : g + 1],
                                         ssum_sb[0:1, g : g + 1])
                    nc.vector.tensor_scalar(
                        out=att_sb[0:1, sl], in0=att_sb[0:1, sl],
                        scalar1=rcp_sb[0:1, g : g + 1], scalar2=None,
                        op0=mybir.AluOpType.mult,
                    )

                # ---- attended[H, G] = sum_cells att * embT ----
                attb_ps = ppool.tile([P, S], F32, tag="psC", space="PSUM", name="attb_ps")
                nc.tensor.matmul(out=attb_ps[:], lhsT=ones_sb[0:1, 0:P],
                                 rhs=att_sb[0:1, :], start=True, stop=True)
                wsum_sb = hpool.tile([P, S], F32, tag="wsum")
                nc.vector.tensor_tensor(
                    out=wsum_sb[:], in0=embT_sb[:], in1=attb_ps[:],
                    op=mybir.AluOpType.mult,
                )
                attd_sb = hpool.tile([P, BL], F32, tag="attd")
                for g in range(BL):
                    nc.vector.tensor_reduce(
                        out=attd_sb[:, g : g + 1],
                        in_=wsum_sb[:, g * NCELLS : (g + 1) * NCELLS],
                        axis=mybir.AxisListType.X, op=mybir.AluOpType.add,
                    )

                # ---- logits = attended.T @ W_fc + b_fc -> [G, OUT] ----
                log_ps = ppool.tile([BL, OUT], F32, tag="psD", space="PSUM", name="log_ps")
                nc.tensor.matmul(out=log_ps[:], lhsT=attd_sb[:, 0:BL],
                                 rhs=wfc_sb[:], start=True, stop=False)
                nc.tensor.matmul(out=log_ps[:], lhsT=ones_sb[0:1, 0:BL],
                                 rhs=bfc_sb[0:1, :], start=False, stop=True,
                                 skip_group_check=True)
                out_sb = hpool.tile([BL, OUT], F32, tag="out")
                nc.vector.tensor_copy(out=out_sb[:], in_=log_ps[:])
                nc.sync.dma_start(out=out[:], in_=out_sb[:])

            if repeat == 1:
                body()
            else:
                with tc.For_i(0, repeat, 1):
                    body()

    nc.compile()
    return nc


def _get(ng: int):
    if ng not in _cache:
        _cache[ng] = _build(ng)
    return _cache[ng]


def _patch_ids(pos: np.ndarray) -> np.ndarray:
    px = np.clip(np.floor(pos[:, 0] / np.float32(PATCH)), 0, NP_ - 1).astype(np.int32)
    py = np.clip(np.floor(pos[:, 1] / np.float32(PATCH)), 0, NP_ - 1).astype(np.int32)
    return px * NP_ + py


def prepare(x_graph_1, x_graph_2, pos_graph_1, pos_graph_2,
            batch_graph_1, batch_graph_2,
            W_emb, b_emb, W_att, b_att, W_fc, b_fc):
    """Compute per-core input maps + the slot size NG. Host-side only."""
    graphs = []
    for x, pos, batch in ((x_graph_1, pos_graph_1, batch_graph_1),
                          (x_graph_2, pos_graph_2, batch_graph_2)):
        x = np.asarray(x, dtype=np.float32)
        pos = np.asarray(pos, dtype=np.float32)
        batch = np.asarray(batch, dtype=np.int32)
        patch = _patch_ids(pos).astype(np.float32)
        bounds = np.searchsorted(batch, np.arange(B + 1)).astype(np.int64)
        if X_MODE == "hilo":
            xhi = x.astype(ml_dtypes.bfloat16)
            xlo = (x - xhi.astype(np.float32)).astype(ml_dtypes.bfloat16)
            x = (xhi, xlo)
        graphs.append((x, patch, bounds))

    span = max(int(np.diff(g[2]).max()) for g in graphs)
    ng = max(-(-span // P) * P, 2 * P)
    ngc = ng // P
    nchunks = BL * ngc
    nrows = BL * ng

    iota_tile = np.tile(np.arange(NCELLS, dtype=np.float32), (P, BUILD_K))
    ident_tile = np.eye(P, dtype=np.float32)
    w_emb = np.asarray(W_emb, dtype=np.float32)
    b_emb2 = np.asarray(b_emb, dtype=np.float32).reshape(H, 1)
    w_att = np.asarray(W_att, dtype=np.float32).reshape(H, 1)
    b_att2 = np.asarray(b_att, dtype=np.float32).reshape(1, 1)
    w_fc = np.asarray(W_fc, dtype=np.float32).reshape(H, OUT)
    b_fc2 = np.asarray(b_fc, dtype=np.float32).reshape(1, OUT)

    in_maps = []
    for k in range(M):
        im = {"iota": iota_tile, "ident": ident_tile, "wemb": w_emb,
              "bemb": b_emb2, "watt": w_att, "batt": b_att2, "wfc": w_fc,
              "bfc": b_fc2}
        for gi, (x, patch, bounds) in enumerate(graphs):
            if X_MODE == "hilo":
                xhi, xlo = x
                xbuf = np.zeros((nrows, 2 * C), dtype=ml_dtypes.bfloat16)
            else:
                xbuf = np.zeros((nrows, C), dtype=np.float32)
            segbuf = np.zeros(nrows, dtype=np.float32)
            for g in range(BL):
                lo = int(bounds[k * BL + g])
                hi = int(bounds[k * BL + g + 1])
                if X_MODE == "hilo":
                    xbuf[g * ng : g * ng + (hi - lo), 0:C] = xhi[lo:hi]
                    xbuf[g * ng : g * ng + (hi - lo), C : 2 * C] = xlo[lo:hi]
                else:
                    xbuf[g * ng : g * ng + (hi - lo)] = x[lo:hi]
                segbuf[g * ng : g * ng + (hi - lo)] = patch[lo:hi]
            im[f"x{gi + 1}"] = np.ascontiguousarray(
                xbuf.reshape(nchunks, P, -1).transpose(1, 0, 2)
                    .reshape(P, -1))
            im[f"seg{gi + 1}"] = np.ascontiguousarray(
                segbuf.reshape(nchunks, P).T)
        in_maps.append(im)
    return ng, in_maps


def kernel(**inputs) -> np.ndarray:
    ng, in_maps = prepare(**inputs)
    nc = _get(ng)
    res = run_bass_kernel_spmd(nc, in_maps, list(range(M)))
    return np.concatenate([res.results[k]["out"] for k in range(M)], axis=0)
